# revision 6
# baseline (speedup 1.0000x reference)
"""Trainium2 Bass kernel for nn_MoESlotDecoder (topk_masking).

Computation (per batch row b, S=64 slots, D=512, C=4, K=16, T=0.01):
    h  = relu(slots @ W1^T + b1)         [B,S,D]
    s  = h @ W2^T + b2                   [B,S,D]
    logits = s @ Wd^T + bd               [B,S,C]
    hk = relu(s @ Wk1^T + bk1)           [B,S,D]
    score = hk @ Wk2^T (+ bk2, dropped: softmax/topk shift-invariant)  [B,S]
    soft = softmax(score/T); hard = top16 mask
    gate = soft*hard / (sum(soft*hard) + 1e-8)
    x = sum_s gate * logits              [B,C]
Outputs: (x, gate, hard).

Strategy:
- Pure data parallel over batch across 8 cores (512 rows/core), weights
  replicated; no collectives.
- Feature-major on-chip dataflow ([din partitions x token free]); slots
  transposed on the PE via identity matmuls.
- fp32 precision via exact hi/lo float32r splits: x = xh + xl with both
  halves fp32r (residual split is exact), matmul as 3 fp32r passes
  (xh*wh + xh*wl + xl*wh) at 1 cyc/row each instead of fp32's 4 cyc/row.
  Verified on HW: rel err 1.3e-7 (same as native fp32) vs 1.5e-4 for raw
  fp32r.  This matters because the top-16 rank gaps go down to 2e-7.
- b2 folded into host-precomputed bk1f = Wk1@b2+bk1 and bdf = Wd@b2+bd.
- Logits head single-pass fp32r (only feeds x, tolerance ~1e-4).
- Top-16 via DVE max8 + match_replace (2 passes), gating with exp/accum on
  ACT, all on [batch-partition x slot] tiles.
"""

import sys

if "/opt/trn_rl_repo" not in sys.path:
    sys.path.insert(0, "/opt/trn_rl_repo")

import numpy as np

B_FULL = 4096
N_CORES = 8
BC = B_FULL // N_CORES  # 512 batch rows per core
S = 64
D = 512
C = 4
KTOP = 16
TT = 512          # tokens per chunk (8 batch rows x 64 slots)
ROWS_PER_CHUNK = TT // S  # 8
INV_T = 100.0     # 1 / temperature
NEG_BIG = -1e30


def build_program(bc=BC, reps=1):
    import concourse.bass as bass
    import concourse.mybir as mybir
    from concourse import bacc
    from concourse.bass import ds, ts
    from concourse.tile import TileContext
    from concourse.masks import make_identity

    FP = mybir.dt.float32
    FPR = mybir.dt.float32r
    AF = mybir.ActivationFunctionType
    OP = mybir.AluOpType
    AX = mybir.AxisListType

    assert bc * S % TT == 0
    nch = bc * S // TT        # chunks per core
    KT = D // 128             # 4 contraction tiles
    GT = (bc + 127) // 128    # gating tiles (batch rows / 128)

    nc = bacc.Bacc("TRN2", target_bir_lowering=False)

    slots_h = nc.dram_tensor("slots", [bc, S, D], FP, kind="ExternalInput")
    W1_h = nc.dram_tensor("W1", [D, D], FP, kind="ExternalInput")
    b1_h = nc.dram_tensor("b1", [D], FP, kind="ExternalInput")
    W2_h = nc.dram_tensor("W2", [D, D], FP, kind="ExternalInput")
    Wd_h = nc.dram_tensor("Wd", [C, D], FP, kind="ExternalInput")
    bdf_h = nc.dram_tensor("bdf", [C], FP, kind="ExternalInput")
    Wk1_h = nc.dram_tensor("Wk1", [D, D], FP, kind="ExternalInput")
    bk1f_h = nc.dram_tensor("bk1f", [D], FP, kind="ExternalInput")
    Wk2_h = nc.dram_tensor("Wk2", [1, D], FP, kind="ExternalInput")

    x_h = nc.dram_tensor("x", [bc, C], FP, kind="ExternalOutput")
    gate_h = nc.dram_tensor("gate", [bc, S], FP, kind="ExternalOutput")
    hard_h = nc.dram_tensor("hard", [bc, S], FP, kind="ExternalOutput")

    flat = slots_h[:, :, :].rearrange("b s d -> (b s) d")  # [bc*S, D]

    with TileContext(nc) as tc:
        with (
            tc.tile_pool(name="const", bufs=1) as constp,
            tc.tile_pool(name="wts", bufs=1) as wp,
            tc.tile_pool(name="ldram", bufs=1, space="DRAM") as dp,
            tc.tile_pool(name="trpsum", bufs=2, space="PSUM") as trp,
            tc.tile_pool(name="mmpsum", bufs=3, space="PSUM") as mmp,
            tc.tile_pool(name="headpsum", bufs=1, space="PSUM") as hp,
        ):
            ident = constp.tile([128, 128], FP)
            make_identity(nc, ident)

            # ---------- weights: load, transpose to [din, dout], split hi/lo ----------
            whs = {}
            wls = {}
            with tc.tile_pool(name="wtmp", bufs=1) as wtmp:
                for name, W_hh in (("w1", W1_h), ("w2", W2_h), ("wk1", Wk1_h)):
                    raw = wtmp.tile([128, KT, D], FP, tag=f"raw_{name}")
                    # raw[p, j, d] = W[j*128 + p, d]
                    nc.sync.dma_start(
                        out=raw, in_=W_hh[:, :].rearrange("(j p) d -> p j d", p=128)
                    )
                    wT = wtmp.tile([128, KT, D], FP, tag=f"wT_{name}")
                    for k in range(KT):
                        for j in range(KT):
                            pt = trp.tile([128, 128], FP, tag="tr", name="pt_w")
                            nc.tensor.transpose(pt, raw[:, j, ts(k, 128)], ident)
                            if (k + j) % 2 == 0:
                                nc.vector.tensor_copy(out=wT[:, k, ts(j, 128)], in_=pt)
                            else:
                                nc.scalar.copy(wT[:, k, ts(j, 128)], pt)
                    wh = wp.tile([128, KT, D], FPR, tag=f"wh_{name}", name=f"wh_{name}")
                    nc.vector.tensor_copy(out=wh, in_=wT)
                    wl = wp.tile([128, KT, D], FPR, tag=f"wl_{name}", name=f"wl_{name}")
                    nc.vector.tensor_sub(wl, wT, wh.bitcast(FP))
                    whs[name] = wh
                    wls[name] = wl

                rawd = wtmp.tile([C, D], FP, tag="raw_wd")
                nc.sync.dma_start(out=rawd, in_=Wd_h[:, :])
                wdT = wtmp.tile([128, KT, C], FP, tag="wdT")
                for k in range(KT):
                    pt = trp.tile([128, C], FP, tag="tr", name="pt_wd")
                    nc.tensor.transpose(pt, rawd[:, ts(k, 128)], ident[:C, :C])
                    nc.vector.tensor_copy(out=wdT[:, k, :], in_=pt)
                wdh = wp.tile([128, KT, C], FPR, name="wdh")
                nc.vector.tensor_copy(out=wdh, in_=wdT)
                wdl = wp.tile([128, KT, C], FPR, name="wdl")
                nc.vector.tensor_sub(wdl, wdT, wdh.bitcast(FP))

                rawk2 = wtmp.tile([1, D], FP, tag="raw_wk2")
                nc.sync.dma_start(out=rawk2, in_=Wk2_h[:, :])
                wk2T = wtmp.tile([128, KT, 1], FP, tag="wk2T")
                for k in range(KT):
                    pt = trp.tile([128, 1], FP, tag="tr", name="pt_wk2")
                    nc.tensor.transpose(pt, rawk2[:, ts(k, 128)], ident[:1, :1])
                    nc.vector.tensor_copy(out=wk2T[:, k, :], in_=pt)
                wk2h = wp.tile([128, KT, 1], FPR, name="wk2h")
                nc.vector.tensor_copy(out=wk2h, in_=wk2T)
                wk2l = wp.tile([128, KT, 1], FPR, name="wk2l")
                nc.vector.tensor_sub(wk2l, wk2T, wk2h.bitcast(FP))

                b1sb = wp.tile([128, KT, 1], FP, tag="b1", name="b1sb")
                nc.sync.dma_start(
                    out=b1sb[:, :, 0], in_=b1_h[:].rearrange("(j p) -> p j", p=128)
                )
                bk1sb = wp.tile([128, KT, 1], FP, tag="bk1", name="bk1sb")
                nc.sync.dma_start(
                    out=bk1sb[:, :, 0], in_=bk1f_h[:].rearrange("(j p) -> p j", p=128)
                )
                bdsb = wp.tile([C, 1], FP, tag="bd", name="bdsb")
                nc.sync.dma_start(
                    out=bdsb, in_=bdf_h[:].rearrange("(c o) -> c o", o=1)
                )

            w1h, w1l = whs["w1"], wls["w1"]
            w2h, w2l = whs["w2"], wls["w2"]
            wk1h, wk1l = whs["wk1"], wls["wk1"]

            # persistent per-core accumulators
            scoresb = wp.tile([128, GT, S], FP, tag="scores")
            logits_dram = dp.tile([C, bc * S], FP)

            with (
                tc.tile_pool(name="io", bufs=2) as iop,
                tc.tile_pool(name="acts", bufs=1) as actp,
                tc.tile_pool(name="scr", bufs=4) as scrp,
                tc.tile_pool(name="small", bufs=2) as sp,
                tc.tile_pool(name="gating", bufs=1) as gp,
            ):
                # ---------- main loop over token chunks ----------
                # (reps>1 repeats the whole compute for wall-clock delta
                #  timing; outputs are rewritten identically each rep)
                import itertools
                for _rep, ct in itertools.product(range(reps), range(nch)):
                    x_tm = iop.tile([128, KT, D], FP, tag="x_tm")
                    # x_tm[p, t, d] = slots_flat[ct*TT + t*128 + p, d]
                    nc.sync.dma_start(
                        out=x_tm,
                        in_=flat[ds(ct * TT, TT), :].rearrange(
                            "(t p) d -> p t d", p=128
                        ),
                    )
                    # transpose to feature-major, split hi/lo while evicting
                    xh = iop.tile([128, KT, TT], FPR, tag="xh")
                    xl = iop.tile([128, KT, TT], FPR, tag="xl")
                    for k in range(KT):
                        pt = trp.tile([128, TT], FP, tag="tr", name="pt_x")
                        for t in range(TT // 128):
                            nc.tensor.transpose(
                                pt[:, ts(t, 128)], x_tm[:, t, ts(k, 128)], ident
                            )
                        nc.scalar.copy(xh[:, k, :], pt)
                        nc.vector.tensor_sub(
                            xl[:, k, :], pt, xh[:, k, :].bitcast(FP)
                        )

                    def layer3(out_hi, out_lo, wh_, wl_, rh_, rl_, bias, relu):
                        """out = act(W @ r + bias); r/W as hi/lo fp32r splits,
                        out written as hi/lo fp32r split (exact residual)."""
                        for j in range(KT):
                            pm = mmp.tile([128, TT], FP, tag="mm", name="pm")
                            n = 0
                            for k in range(KT):
                                for lh_ap, rh_ap in (
                                    (wh_[:, k, ts(j, 128)], rh_[:, k, :]),
                                    (wl_[:, k, ts(j, 128)], rh_[:, k, :]),
                                    (wh_[:, k, ts(j, 128)], rl_[:, k, :]),
                                ):
                                    nc.tensor.matmul(
                                        pm, lhsT=lh_ap, rhs=rh_ap,
                                        start=(n == 0), stop=(n == 3 * KT - 1),
                                    )
                                    n += 1
                            if relu:
                                scratch = scrp.tile(
                                    [128, TT], FP, tag="scratch", name="scratch"
                                )
                                nc.scalar.activation(
                                    out=scratch, in_=pm, func=AF.Relu,
                                    bias=bias[:, j, :],
                                )
                                nc.vector.tensor_copy(out=out_hi[:, j, :], in_=scratch)
                                nc.vector.tensor_sub(
                                    out_lo[:, j, :], scratch,
                                    out_hi[:, j, :].bitcast(FP),
                                )
                            else:
                                nc.scalar.copy(out_hi[:, j, :], pm)
                                nc.vector.tensor_sub(
                                    out_lo[:, j, :], pm, out_hi[:, j, :].bitcast(FP)
                                )

                    # layer 1: h = relu(W1 @ x + b1)
                    hh = actp.tile([128, KT, TT], FPR, tag="hh")
                    hl = actp.tile([128, KT, TT], FPR, tag="hl")
                    layer3(hh, hl, w1h, w1l, xh, xl, b1sb, relu=True)

                    # layer 2: s = W2 @ h   (b2 folded into bk1f/bdf)
                    sh = actp.tile([128, KT, TT], FPR, tag="sh")
                    sl = actp.tile([128, KT, TT], FPR, tag="sl")
                    layer3(sh, sl, w2h, w2l, hh, hl, None, relu=False)

                    # layer 3: hk = relu(Wk1 @ s + bk1f)
                    hkh = actp.tile([128, KT, TT], FPR, tag="hkh")
                    hkl = actp.tile([128, KT, TT], FPR, tag="hkl")
                    layer3(hkh, hkl, wk1h, wk1l, sh, sl, bk1sb, relu=True)

                    # logits head (3-pass split): [C, TT] = Wd @ s + bdf
                    pl = hp.tile([C, TT], FP, tag="lg", name="pl")
                    n = 0
                    for k in range(KT):
                        for lh_ap, rh_ap in (
                            (wdh[:, k, :], sh[:, k, :]),
                            (wdl[:, k, :], sh[:, k, :]),
                            (wdh[:, k, :], sl[:, k, :]),
                        ):
                            nc.tensor.matmul(
                                pl, lhsT=lh_ap, rhs=rh_ap,
                                start=(n == 0), stop=(n == 3 * KT - 1),
                            )
                            n += 1
                    lg_sb = sp.tile([C, TT], FP, tag="lg_sb")
                    nc.scalar.activation(out=lg_sb, in_=pl, func=AF.Identity, bias=bdsb)
                    nc.sync.dma_start(out=logits_dram[:, ds(ct * TT, TT)], in_=lg_sb)

                    # keep-score head (3-pass split): [1, TT] = Wk2 @ hk
                    psc = hp.tile([1, TT], FP, tag="sc", name="psc")
                    n = 0
                    for k in range(KT):
                        for lh_ap, rh_ap in (
                            (wk2h[:, k, :], hkh[:, k, :]),
                            (wk2l[:, k, :], hkh[:, k, :]),
                            (wk2h[:, k, :], hkl[:, k, :]),
                        ):
                            nc.tensor.matmul(
                                psc, lhsT=lh_ap, rhs=rh_ap,
                                start=(n == 0), stop=(n == 3 * KT - 1),
                            )
                            n += 1
                    sc_sb = sp.tile([1, TT], FP, tag="sc_sb")
                    nc.scalar.copy(sc_sb, psc)
                    # reshape [1, 512] -> [8 rows, 64 slots] at batch-row partitions
                    base = (ct * ROWS_PER_CHUNK) % 128
                    g1 = (ct * ROWS_PER_CHUNK) // 128
                    nc.sync.dma_start(
                        out=scoresb[ds(base, ROWS_PER_CHUNK), g1, :], in_=sc_sb
                    )

                # ---------- gating: top-16 + peaked softmax + weighted logits ----------
                for g in range(GT):
                    pcnt = min(128, bc - g * 128)
                    sc_g = scoresb[:pcnt, g, :]                      # [pcnt, 64]
                    m8a = gp.tile([128, 8], FP, tag="m8a", name="m8a")[:pcnt, :]
                    nc.vector.max(out=m8a, in_=sc_g)                 # ranks 1-8
                    scratch = gp.tile([128, S], FP, tag="scratch", name="scr_g")[:pcnt, :]
                    nc.vector.match_replace(
                        out=scratch, in_to_replace=m8a, in_values=sc_g,
                        imm_value=NEG_BIG,
                    )
                    m8b = gp.tile([128, 8], FP, tag="m8b", name="m8b")[:pcnt, :]
                    nc.vector.max(out=m8b, in_=scratch)              # ranks 9-16
                    hard_g = gp.tile([128, S], FP, tag="hard", name="hard_t")[:pcnt, :]
                    nc.vector.tensor_scalar(
                        out=hard_g, in0=sc_g, scalar1=m8b[:, 7:8], scalar2=None,
                        op0=OP.is_ge,
                    )
                    negm = gp.tile([128, 1], FP, tag="negm", name="negm")[:pcnt, :]
                    nc.vector.tensor_scalar_mul(negm, m8a[:, 0:1], -INV_T)
                    e_g = gp.tile([128, S], FP, tag="e", name="e_t")[:pcnt, :]
                    esum = gp.tile([128, 1], FP, tag="esum", name="esum")[:pcnt, :]
                    nc.scalar.activation(
                        out=e_g, in_=sc_g, func=AF.Exp, bias=negm, scale=INV_T,
                        accum_out=esum,
                    )
                    gu = gp.tile([128, S], FP, tag="gu", name="gu")[:pcnt, :]
                    nc.vector.tensor_tensor(out=gu, in0=e_g, in1=hard_g, op=OP.mult)
                    ssum = gp.tile([128, 1], FP, tag="ssum", name="ssum")[:pcnt, :]
                    nc.vector.reduce_sum(out=ssum, in_=gu, axis=AX.X)
                    denom = gp.tile([128, 1], FP, tag="denom", name="denom")[:pcnt, :]
                    nc.vector.tensor_scalar(
                        out=denom, in0=esum, scalar1=1e-8, scalar2=None, op0=OP.mult
                    )
                    nc.vector.tensor_add(denom, denom, ssum)
                    rec = gp.tile([128, 1], FP, tag="rec", name="rec")[:pcnt, :]
                    nc.vector.reciprocal(rec, denom)
                    gate_g = gp.tile([128, S], FP, tag="gate", name="gate_t")[:pcnt, :]
                    nc.vector.tensor_scalar_mul(gate_g, gu, rec)

                    nc.sync.dma_start(out=gate_h[ds(g * 128, pcnt), :], in_=gate_g)
                    nc.sync.dma_start(out=hard_h[ds(g * 128, pcnt), :], in_=hard_g)

                    # x[b, c] = sum_s gate[b, s] * logits[c, b*S + s]
                    xo = gp.tile([128, C], FP, tag="xo", name="xo")[:pcnt, :]
                    for c in range(C):
                        lc = gp.tile([128, S], FP, tag="lc", name="lc")[:pcnt, :]
                        nc.sync.dma_start(
                            out=lc, in_=logits_dram[c, ds(g * 128 * S, pcnt * S)]
                        )
                        tmp = gp.tile([128, S], FP, tag="xtmp", name="xtmp")[:pcnt, :]
                        nc.vector.tensor_tensor(out=tmp, in0=gate_g, in1=lc, op=OP.mult)
                        nc.vector.reduce_sum(out=xo[:, c : c + 1], in_=tmp, axis=AX.X)
                    nc.sync.dma_start(out=x_h[ds(g * 128, pcnt), :], in_=xo)

    nc.compile()
    return nc


_NC_CACHE = {}


def _get_program(bc):
    if bc not in _NC_CACHE:
        _NC_CACHE[bc] = build_program(bc)
    return _NC_CACHE[bc]


def make_weight_inputs(W1, b1, W2, b2, Wd, bd, Wk1, bk1, Wk2):
    """Device weight dict with b2 folded into downstream biases (fp64 fold)."""
    bk1f = (Wk1.astype(np.float64) @ b2.astype(np.float64) + bk1).astype(np.float32)
    bdf = (Wd.astype(np.float64) @ b2.astype(np.float64) + bd).astype(np.float32)
    return {
        "W1": np.ascontiguousarray(W1, np.float32),
        "b1": np.ascontiguousarray(b1, np.float32),
        "W2": np.ascontiguousarray(W2, np.float32),
        "Wd": np.ascontiguousarray(Wd, np.float32),
        "bdf": np.ascontiguousarray(bdf, np.float32),
        "Wk1": np.ascontiguousarray(Wk1, np.float32),
        "bk1f": np.ascontiguousarray(bk1f, np.float32),
        "Wk2": np.ascontiguousarray(Wk2, np.float32),
    }


def kernel(slots, W1, b1, W2, b2, Wd, bd, Wk1, bk1, Wk2, bk2):
    from concourse.bass_utils import run_bass_kernel_spmd

    B = slots.shape[0]
    assert B % N_CORES == 0
    bc = B // N_CORES
    nc = _get_program(bc)

    weights = make_weight_inputs(W1, b1, W2, b2, Wd, bd, Wk1, bk1, Wk2)
    in_maps = []
    for i in range(N_CORES):
        m = dict(weights)
        m["slots"] = np.ascontiguousarray(slots[i * bc : (i + 1) * bc], np.float32)
        in_maps.append(m)

    res = run_bass_kernel_spmd(nc, in_maps, core_ids=list(range(N_CORES)))
    x = np.concatenate([r["x"] for r in res.results], axis=0)
    gate = np.concatenate([r["gate"] for r in res.results], axis=0)
    hard = np.concatenate([r["hard"] for r in res.results], axis=0)
    return x, gate, hard


if __name__ == "__main__":
    import time

    t0 = time.time()
    nc = build_program(int(sys.argv[1]) if len(sys.argv) > 1 else BC)
    print(f"traced in {time.time() - t0:.1f}s")


# revision 10
# speedup vs baseline: 1.0000x; 1.0000x over previous
"""Trainium2 Bass kernel for nn_MoESlotDecoder (topk_masking).

Computation (per batch row b, S=64 slots, D=512, C=4, K=16, T=0.01):
    h  = relu(slots @ W1^T + b1)         [B,S,D]
    s  = h @ W2^T + b2                   [B,S,D]
    logits = s @ Wd^T + bd               [B,S,C]
    hk = relu(s @ Wk1^T + bk1)           [B,S,D]
    score = hk @ Wk2^T (+ bk2, dropped: softmax/topk shift-invariant)  [B,S]
    soft = softmax(score/T); hard = top16 mask
    gate = soft*hard / (sum(soft*hard) + 1e-8)
    x = sum_s gate * logits              [B,C]
Outputs: (x, gate, hard).

Strategy:
- Pure data parallel over batch across 8 cores (512 rows/core), weights
  replicated; no collectives.
- Feature-major on-chip dataflow ([din partitions x token free]); slots
  transposed on the PE via identity matmuls.
- fp32 precision via exact hi/lo float32r splits: x = xh + xl with both
  halves fp32r (residual split is exact), matmul as 3 fp32r passes
  (xh*wh + xh*wl + xl*wh) at 1 cyc/row each instead of fp32's 4 cyc/row.
  Verified on HW: rel err 1.3e-7 (same as native fp32) vs 1.5e-4 for raw
  fp32r.  This matters because the top-16 rank gaps go down to 2e-7.
- b2 folded into host-precomputed bk1f = Wk1@b2+bk1 and bdf = Wd@b2+bd.
- Logits head single-pass fp32r (only feeds x, tolerance ~1e-4).
- Top-16 via DVE max8 + match_replace (2 passes), gating with exp/accum on
  ACT, all on [batch-partition x slot] tiles.
"""

import sys

if "/opt/trn_rl_repo" not in sys.path:
    sys.path.insert(0, "/opt/trn_rl_repo")

import numpy as np

B_FULL = 4096
N_CORES = 8
BC = B_FULL // N_CORES  # 512 batch rows per core
S = 64
D = 512
C = 4
KTOP = 16
TT = 512          # tokens per chunk (8 batch rows x 64 slots)
ROWS_PER_CHUNK = TT // S  # 8
INV_T = 100.0     # 1 / temperature
NEG_BIG = -1e30


def build_program(bc=BC, reps=1):
    import concourse.bass as bass
    import concourse.mybir as mybir
    from concourse import bacc
    from concourse.bass import ds, ts
    from concourse.tile import TileContext
    from concourse.masks import make_identity

    FP = mybir.dt.float32
    FPR = mybir.dt.float32r
    AF = mybir.ActivationFunctionType
    OP = mybir.AluOpType
    AX = mybir.AxisListType

    assert bc * S % TT == 0
    nch = bc * S // TT        # chunks per core
    KT = D // 128             # 4 contraction tiles
    GT = (bc + 127) // 128    # gating tiles (batch rows / 128)

    nc = bacc.Bacc("TRN2", target_bir_lowering=False)

    slots_h = nc.dram_tensor("slots", [bc, S, D], FP, kind="ExternalInput")
    W1_h = nc.dram_tensor("W1", [D, D], FP, kind="ExternalInput")
    b1_h = nc.dram_tensor("b1", [D], FP, kind="ExternalInput")
    W2_h = nc.dram_tensor("W2", [D, D], FP, kind="ExternalInput")
    Wd_h = nc.dram_tensor("Wd", [C, D], FP, kind="ExternalInput")
    bdf_h = nc.dram_tensor("bdf", [C], FP, kind="ExternalInput")
    Wk1_h = nc.dram_tensor("Wk1", [D, D], FP, kind="ExternalInput")
    bk1f_h = nc.dram_tensor("bk1f", [D], FP, kind="ExternalInput")
    Wk2_h = nc.dram_tensor("Wk2", [1, D], FP, kind="ExternalInput")

    x_h = nc.dram_tensor("x", [bc, C], FP, kind="ExternalOutput")
    gate_h = nc.dram_tensor("gate", [bc, S], FP, kind="ExternalOutput")
    hard_h = nc.dram_tensor("hard", [bc, S], FP, kind="ExternalOutput")

    flat = slots_h[:, :, :].rearrange("b s d -> (b s) d")  # [bc*S, D]

    with TileContext(nc) as tc:
        with (
            tc.tile_pool(name="const", bufs=1) as constp,
            tc.tile_pool(name="wts", bufs=1) as wp,
            tc.tile_pool(name="ldram", bufs=1, space="DRAM") as dp,
            tc.tile_pool(name="trpsum", bufs=2, space="PSUM") as trp,
            tc.tile_pool(name="mmpsum", bufs=4, space="PSUM") as mmp,
            tc.tile_pool(name="headpsum", bufs=1, space="PSUM") as hp,
        ):
            ident = constp.tile([128, 128], FP)
            make_identity(nc, ident)

            # ---------- weights: load, transpose to [din, dout], split hi/lo ----------
            whs = {}
            wls = {}
            with tc.tile_pool(name="wtmp", bufs=1) as wtmp:
                for name, W_hh in (("w1", W1_h), ("w2", W2_h), ("wk1", Wk1_h)):
                    raw = wtmp.tile([128, KT, D], FP, tag=f"raw_{name}")
                    # raw[p, j, d] = W[j*128 + p, d]
                    nc.sync.dma_start(
                        out=raw, in_=W_hh[:, :].rearrange("(j p) d -> p j d", p=128)
                    )
                    wT = wtmp.tile([128, KT, D], FP, tag=f"wT_{name}")
                    for k in range(KT):
                        for j in range(KT):
                            pt = trp.tile([128, 128], FP, tag="tr", name="pt_w")
                            nc.tensor.transpose(pt, raw[:, j, ts(k, 128)], ident)
                            if (k + j) % 2 == 0:
                                nc.vector.tensor_copy(out=wT[:, k, ts(j, 128)], in_=pt)
                            else:
                                nc.scalar.copy(wT[:, k, ts(j, 128)], pt)
                    wh = wp.tile([128, KT, D], FPR, tag=f"wh_{name}", name=f"wh_{name}")
                    nc.vector.tensor_copy(out=wh, in_=wT)
                    wl = wp.tile([128, KT, D], FPR, tag=f"wl_{name}", name=f"wl_{name}")
                    nc.vector.tensor_sub(wl, wT, wh.bitcast(FP))
                    whs[name] = wh
                    wls[name] = wl

                rawd = wtmp.tile([C, D], FP, tag="raw_wd")
                nc.sync.dma_start(out=rawd, in_=Wd_h[:, :])
                wdT = wtmp.tile([128, KT, C], FP, tag="wdT")
                for k in range(KT):
                    pt = trp.tile([128, C], FP, tag="tr", name="pt_wd")
                    nc.tensor.transpose(pt, rawd[:, ts(k, 128)], ident[:C, :C])
                    nc.vector.tensor_copy(out=wdT[:, k, :], in_=pt)
                wdh = wp.tile([128, KT, C], FPR, name="wdh")
                nc.vector.tensor_copy(out=wdh, in_=wdT)
                wdl = wp.tile([128, KT, C], FPR, name="wdl")
                nc.vector.tensor_sub(wdl, wdT, wdh.bitcast(FP))

                rawk2 = wtmp.tile([1, D], FP, tag="raw_wk2")
                nc.sync.dma_start(out=rawk2, in_=Wk2_h[:, :])
                wk2T = wtmp.tile([128, KT, 1], FP, tag="wk2T")
                for k in range(KT):
                    pt = trp.tile([128, 1], FP, tag="tr", name="pt_wk2")
                    nc.tensor.transpose(pt, rawk2[:, ts(k, 128)], ident[:1, :1])
                    nc.vector.tensor_copy(out=wk2T[:, k, :], in_=pt)
                wk2h = wp.tile([128, KT, 1], FPR, name="wk2h")
                nc.vector.tensor_copy(out=wk2h, in_=wk2T)
                wk2l = wp.tile([128, KT, 1], FPR, name="wk2l")
                nc.vector.tensor_sub(wk2l, wk2T, wk2h.bitcast(FP))

                b1sb = wp.tile([128, KT, 1], FP, tag="b1", name="b1sb")
                nc.sync.dma_start(
                    out=b1sb[:, :, 0], in_=b1_h[:].rearrange("(j p) -> p j", p=128)
                )
                bk1sb = wp.tile([128, KT, 1], FP, tag="bk1", name="bk1sb")
                nc.sync.dma_start(
                    out=bk1sb[:, :, 0], in_=bk1f_h[:].rearrange("(j p) -> p j", p=128)
                )
                bdsb = wp.tile([C, 1], FP, tag="bd", name="bdsb")
                nc.sync.dma_start(
                    out=bdsb, in_=bdf_h[:].rearrange("(c o) -> c o", o=1)
                )

            w1h, w1l = whs["w1"], wls["w1"]
            w2h, w2l = whs["w2"], wls["w2"]
            wk1h, wk1l = whs["wk1"], wls["wk1"]

            # persistent per-core accumulators
            scoresb = wp.tile([128, GT, S], FP, tag="scores")
            logits_dram = dp.tile([C, bc * S], FP)

            with (
                tc.tile_pool(name="io", bufs=2) as iop,
                tc.tile_pool(name="acts", bufs=1) as actp,
                tc.tile_pool(name="scr", bufs=4) as scrp,
                tc.tile_pool(name="small", bufs=2) as sp,
                tc.tile_pool(name="gating", bufs=1) as gp,
            ):
                # ---------- gating: top-16 + peaked softmax + weighted logits ----------
                def emit_gating(g):
                    pcnt = min(128, bc - g * 128)
                    sc_g = scoresb[:pcnt, g, :]                      # [pcnt, 64]
                    m8a = gp.tile([128, 8], FP, tag="m8a", name="m8a")[:pcnt, :]
                    nc.vector.max(out=m8a, in_=sc_g)                 # ranks 1-8
                    scratch = gp.tile([128, S], FP, tag="scratch", name="scr_g")[:pcnt, :]
                    nc.vector.match_replace(
                        out=scratch, in_to_replace=m8a, in_values=sc_g,
                        imm_value=NEG_BIG,
                    )
                    m8b = gp.tile([128, 8], FP, tag="m8b", name="m8b")[:pcnt, :]
                    nc.vector.max(out=m8b, in_=scratch)              # ranks 9-16
                    hard_g = gp.tile([128, S], FP, tag="hard", name="hard_t")[:pcnt, :]
                    nc.vector.tensor_scalar(
                        out=hard_g, in0=sc_g, scalar1=m8b[:, 7:8], scalar2=None,
                        op0=OP.is_ge,
                    )
                    negm = gp.tile([128, 1], FP, tag="negm", name="negm")[:pcnt, :]
                    nc.vector.tensor_scalar_mul(negm, m8a[:, 0:1], -INV_T)
                    e_g = gp.tile([128, S], FP, tag="e", name="e_t")[:pcnt, :]
                    esum = gp.tile([128, 1], FP, tag="esum", name="esum")[:pcnt, :]
                    nc.scalar.activation(
                        out=e_g, in_=sc_g, func=AF.Exp, bias=negm, scale=INV_T,
                        accum_out=esum,
                    )
                    gu = gp.tile([128, S], FP, tag="gu", name="gu")[:pcnt, :]
                    nc.vector.tensor_tensor(out=gu, in0=e_g, in1=hard_g, op=OP.mult)
                    ssum = gp.tile([128, 1], FP, tag="ssum", name="ssum")[:pcnt, :]
                    nc.vector.reduce_sum(out=ssum, in_=gu, axis=AX.X)
                    denom = gp.tile([128, 1], FP, tag="denom", name="denom")[:pcnt, :]
                    nc.vector.tensor_scalar(
                        out=denom, in0=esum, scalar1=1e-8, scalar2=None, op0=OP.mult
                    )
                    nc.vector.tensor_add(denom, denom, ssum)
                    rec = gp.tile([128, 1], FP, tag="rec", name="rec")[:pcnt, :]
                    nc.vector.reciprocal(rec, denom)
                    gate_g = gp.tile([128, S], FP, tag="gate", name="gate_t")[:pcnt, :]
                    nc.vector.tensor_scalar_mul(gate_g, gu, rec)

                    nc.sync.dma_start(out=gate_h[ds(g * 128, pcnt), :], in_=gate_g)
                    nc.sync.dma_start(out=hard_h[ds(g * 128, pcnt), :], in_=hard_g)

                    # x[b, c] = sum_s gate[b, s] * logits[c, b*S + s]
                    xo = gp.tile([128, C], FP, tag="xo", name="xo")[:pcnt, :]
                    for c in range(C):
                        lc = gp.tile([128, S], FP, tag="lc", name="lc")[:pcnt, :]
                        nc.sync.dma_start(
                            out=lc, in_=logits_dram[c, ds(g * 128 * S, pcnt * S)]
                        )
                        tmp = gp.tile([128, S], FP, tag="xtmp", name="xtmp")[:pcnt, :]
                        nc.vector.tensor_tensor(out=tmp, in0=gate_g, in1=lc, op=OP.mult)
                        nc.vector.reduce_sum(out=xo[:, c : c + 1], in_=tmp, axis=AX.X)
                    nc.sync.dma_start(out=x_h[ds(g * 128, pcnt), :], in_=xo)

                # ---------- main loop over token chunks ----------
                # (reps>1 repeats the whole compute for wall-clock delta
                #  timing; outputs are rewritten identically each rep)
                import itertools
                for _rep, ct in itertools.product(range(reps), range(nch)):
                    x_tm = iop.tile([128, KT, D], FP, tag="x_tm")
                    # x_tm[p, t, d] = slots_flat[ct*TT + t*128 + p, d]
                    nc.sync.dma_start(
                        out=x_tm,
                        in_=flat[ds(ct * TT, TT), :].rearrange(
                            "(t p) d -> p t d", p=128
                        ),
                    )
                    # transpose to feature-major, split hi/lo while evicting
                    xh = iop.tile([128, KT, TT], FPR, tag="xh")
                    xl = iop.tile([128, KT, TT], FPR, tag="xl")
                    for k in range(KT):
                        pt = trp.tile([128, TT], FP, tag="tr", name="pt_x")
                        for t in range(TT // 128):
                            nc.tensor.transpose(
                                pt[:, ts(t, 128)], x_tm[:, t, ts(k, 128)], ident
                            )
                        nc.scalar.copy(xh[:, k, :], pt)
                        nc.vector.tensor_sub(
                            xl[:, k, :], pt, xh[:, k, :].bitcast(FP)
                        )

                    def layer3(out_hi, out_lo, wh_, wl_, rh_, rl_, bias, relu):
                        """out = act(W @ r + bias); r/W as hi/lo fp32r splits,
                        out written as hi/lo fp32r split (exact residual)."""
                        for j in range(KT):
                            pm = mmp.tile([128, TT], FP, tag="mm", name="pm")
                            n = 0
                            for k in range(KT):
                                for lh_ap, rh_ap in (
                                    (wh_[:, k, ts(j, 128)], rh_[:, k, :]),
                                    (wl_[:, k, ts(j, 128)], rh_[:, k, :]),
                                    (wh_[:, k, ts(j, 128)], rl_[:, k, :]),
                                ):
                                    nc.tensor.matmul(
                                        pm, lhsT=lh_ap, rhs=rh_ap,
                                        start=(n == 0), stop=(n == 3 * KT - 1),
                                    )
                                    n += 1
                            if relu:
                                scratch = scrp.tile(
                                    [128, TT], FP, tag="scratch", name="scratch"
                                )
                                nc.scalar.activation(
                                    out=scratch, in_=pm, func=AF.Relu,
                                    bias=bias[:, j, :],
                                )
                                nc.vector.tensor_copy(out=out_hi[:, j, :], in_=scratch)
                                nc.vector.tensor_sub(
                                    out_lo[:, j, :], scratch,
                                    out_hi[:, j, :].bitcast(FP),
                                )
                            else:
                                nc.scalar.copy(out_hi[:, j, :], pm)
                                nc.vector.tensor_sub(
                                    out_lo[:, j, :], pm, out_hi[:, j, :].bitcast(FP)
                                )

                    # layer 1: h = relu(W1 @ x + b1)
                    hh = actp.tile([128, KT, TT], FPR, tag="hh")
                    hl = actp.tile([128, KT, TT], FPR, tag="hl")
                    layer3(hh, hl, w1h, w1l, xh, xl, b1sb, relu=True)

                    # layer 2: s = W2 @ h   (b2 folded into bk1f/bdf)
                    sh = actp.tile([128, KT, TT], FPR, tag="sh")
                    sl = actp.tile([128, KT, TT], FPR, tag="sl")
                    layer3(sh, sl, w2h, w2l, hh, hl, None, relu=False)

                    # layer 3: hk = relu(Wk1 @ s + bk1f)
                    hkh = actp.tile([128, KT, TT], FPR, tag="hkh")
                    hkl = actp.tile([128, KT, TT], FPR, tag="hkl")
                    layer3(hkh, hkl, wk1h, wk1l, sh, sl, bk1sb, relu=True)

                    # logits head (3-pass split): [C, TT] = Wd @ s + bdf
                    pl = hp.tile([C, TT], FP, tag="lg", name="pl")
                    n = 0
                    for k in range(KT):
                        for lh_ap, rh_ap in (
                            (wdh[:, k, :], sh[:, k, :]),
                            (wdl[:, k, :], sh[:, k, :]),
                            (wdh[:, k, :], sl[:, k, :]),
                        ):
                            nc.tensor.matmul(
                                pl, lhsT=lh_ap, rhs=rh_ap,
                                start=(n == 0), stop=(n == 3 * KT - 1),
                            )
                            n += 1
                    lg_sb = sp.tile([C, TT], FP, tag="lg_sb")
                    nc.scalar.activation(out=lg_sb, in_=pl, func=AF.Identity, bias=bdsb)
                    nc.sync.dma_start(out=logits_dram[:, ds(ct * TT, TT)], in_=lg_sb)

                    # keep-score head (3-pass split): [1, TT] = Wk2 @ hk
                    psc = hp.tile([1, TT], FP, tag="sc", name="psc")
                    n = 0
                    for k in range(KT):
                        for lh_ap, rh_ap in (
                            (wk2h[:, k, :], hkh[:, k, :]),
                            (wk2l[:, k, :], hkh[:, k, :]),
                            (wk2h[:, k, :], hkl[:, k, :]),
                        ):
                            nc.tensor.matmul(
                                psc, lhsT=lh_ap, rhs=rh_ap,
                                start=(n == 0), stop=(n == 3 * KT - 1),
                            )
                            n += 1
                    sc_sb = sp.tile([1, TT], FP, tag="sc_sb")
                    nc.scalar.copy(sc_sb, psc)
                    # reshape [1, 512] -> [8 rows, 64 slots] at batch-row partitions
                    base = (ct * ROWS_PER_CHUNK) % 128
                    g1 = (ct * ROWS_PER_CHUNK) // 128
                    nc.sync.dma_start(
                        out=scoresb[ds(base, ROWS_PER_CHUNK), g1, :], in_=sc_sb
                    )
                    rows_done = (ct + 1) * ROWS_PER_CHUNK
                    if rows_done % 128 == 0 and _rep == reps - 1:
                        emit_gating(rows_done // 128 - 1)


                for g in range((bc * S // TT) * ROWS_PER_CHUNK // 128, GT):
                    emit_gating(g)

    nc.compile()
    return nc


_NC_CACHE = {}


def _get_program(bc):
    if bc not in _NC_CACHE:
        _NC_CACHE[bc] = build_program(bc)
    return _NC_CACHE[bc]


def make_weight_inputs(W1, b1, W2, b2, Wd, bd, Wk1, bk1, Wk2):
    """Device weight dict with b2 folded into downstream biases (fp64 fold)."""
    bk1f = (Wk1.astype(np.float64) @ b2.astype(np.float64) + bk1).astype(np.float32)
    bdf = (Wd.astype(np.float64) @ b2.astype(np.float64) + bd).astype(np.float32)
    return {
        "W1": np.ascontiguousarray(W1, np.float32),
        "b1": np.ascontiguousarray(b1, np.float32),
        "W2": np.ascontiguousarray(W2, np.float32),
        "Wd": np.ascontiguousarray(Wd, np.float32),
        "bdf": np.ascontiguousarray(bdf, np.float32),
        "Wk1": np.ascontiguousarray(Wk1, np.float32),
        "bk1f": np.ascontiguousarray(bk1f, np.float32),
        "Wk2": np.ascontiguousarray(Wk2, np.float32),
    }


def kernel(slots, W1, b1, W2, b2, Wd, bd, Wk1, bk1, Wk2, bk2):
    from concourse.bass_utils import run_bass_kernel_spmd

    B = slots.shape[0]
    assert B % N_CORES == 0
    bc = B // N_CORES
    nc = _get_program(bc)

    weights = make_weight_inputs(W1, b1, W2, b2, Wd, bd, Wk1, bk1, Wk2)
    in_maps = []
    for i in range(N_CORES):
        m = dict(weights)
        m["slots"] = np.ascontiguousarray(slots[i * bc : (i + 1) * bc], np.float32)
        in_maps.append(m)

    res = run_bass_kernel_spmd(nc, in_maps, core_ids=list(range(N_CORES)))
    x = np.concatenate([r["x"] for r in res.results], axis=0)
    gate = np.concatenate([r["gate"] for r in res.results], axis=0)
    hard = np.concatenate([r["hard"] for r in res.results], axis=0)
    return x, gate, hard


if __name__ == "__main__":
    import time

    t0 = time.time()
    nc = build_program(int(sys.argv[1]) if len(sys.argv) > 1 else BC)
    print(f"traced in {time.time() - t0:.1f}s")


# revision 11
# speedup vs baseline: 1.0000x; 1.0000x over previous
"""Trainium2 Bass kernel for nn_MoESlotDecoder (topk_masking).

Computation (per batch row b, S=64 slots, D=512, C=4, K=16, T=0.01):
    h  = relu(slots @ W1^T + b1)         [B,S,D]
    s  = h @ W2^T + b2                   [B,S,D]
    logits = s @ Wd^T + bd               [B,S,C]
    hk = relu(s @ Wk1^T + bk1)           [B,S,D]
    score = hk @ Wk2^T (+ bk2, dropped: softmax/topk shift-invariant)  [B,S]
    soft = softmax(score/T); hard = top16 mask
    gate = soft*hard / (sum(soft*hard) + 1e-8)
    x = sum_s gate * logits              [B,C]
Outputs: (x, gate, hard).

Strategy:
- Pure data parallel over batch across 8 cores (512 rows/core), weights
  replicated; no collectives.
- Feature-major on-chip dataflow ([din partitions x token free]); slots
  transposed on the PE via identity matmuls.
- fp32 precision via exact hi/lo float32r splits: x = xh + xl with both
  halves fp32r (residual split is exact), matmul as 3 fp32r passes
  (xh*wh + xh*wl + xl*wh) at 1 cyc/row each instead of fp32's 4 cyc/row.
  Verified on HW: rel err 1.3e-7 (same as native fp32) vs 1.5e-4 for raw
  fp32r.  This matters because the top-16 rank gaps go down to 2e-7.
- b2 folded into host-precomputed bk1f = Wk1@b2+bk1 and bdf = Wd@b2+bd.
- Logits head single-pass fp32r (only feeds x, tolerance ~1e-4).
- Top-16 via DVE max8 + match_replace (2 passes), gating with exp/accum on
  ACT, all on [batch-partition x slot] tiles.
"""

import sys

if "/opt/trn_rl_repo" not in sys.path:
    sys.path.insert(0, "/opt/trn_rl_repo")

import numpy as np

B_FULL = 4096
N_CORES = 8
BC = B_FULL // N_CORES  # 512 batch rows per core
S = 64
D = 512
C = 4
KTOP = 16
TT = 512          # tokens per chunk (8 batch rows x 64 slots)
ROWS_PER_CHUNK = TT // S  # 8
INV_T = 100.0     # 1 / temperature
NEG_BIG = -1e30


def build_program(bc=BC, reps=1):
    import concourse.bass as bass
    import concourse.mybir as mybir
    from concourse import bacc
    from concourse.bass import ds, ts
    from concourse.tile import TileContext
    from concourse.masks import make_identity

    FP = mybir.dt.float32
    FPR = mybir.dt.float32r
    AF = mybir.ActivationFunctionType
    OP = mybir.AluOpType
    AX = mybir.AxisListType

    assert bc * S % TT == 0
    nch = bc * S // TT        # chunks per core
    KT = D // 128             # 4 contraction tiles
    GT = (bc + 127) // 128    # gating tiles (batch rows / 128)

    nc = bacc.Bacc("TRN2", target_bir_lowering=False)

    slots_h = nc.dram_tensor("slots", [bc, S, D], FP, kind="ExternalInput")
    W1_h = nc.dram_tensor("W1", [D, D], FP, kind="ExternalInput")
    b1_h = nc.dram_tensor("b1", [D], FP, kind="ExternalInput")
    W2_h = nc.dram_tensor("W2", [D, D], FP, kind="ExternalInput")
    Wd_h = nc.dram_tensor("Wd", [C, D], FP, kind="ExternalInput")
    bdf_h = nc.dram_tensor("bdf", [C], FP, kind="ExternalInput")
    Wk1_h = nc.dram_tensor("Wk1", [D, D], FP, kind="ExternalInput")
    bk1f_h = nc.dram_tensor("bk1f", [D], FP, kind="ExternalInput")
    Wk2_h = nc.dram_tensor("Wk2", [1, D], FP, kind="ExternalInput")

    x_h = nc.dram_tensor("x", [bc, C], FP, kind="ExternalOutput")
    gate_h = nc.dram_tensor("gate", [bc, S], FP, kind="ExternalOutput")
    hard_h = nc.dram_tensor("hard", [bc, S], FP, kind="ExternalOutput")

    flat = slots_h[:, :, :].rearrange("b s d -> (b s) d")  # [bc*S, D]

    with TileContext(nc) as tc:
        with (
            tc.tile_pool(name="const", bufs=1) as constp,
            tc.tile_pool(name="wts", bufs=1) as wp,
            tc.tile_pool(name="ldram", bufs=1, space="DRAM") as dp,
            tc.tile_pool(name="trpsum", bufs=2, space="PSUM") as trp,
            tc.tile_pool(name="mmpsum", bufs=4, space="PSUM") as mmp,
            tc.tile_pool(name="headpsum", bufs=1, space="PSUM") as hp,
        ):
            ident = constp.tile([128, 128], FP)
            make_identity(nc, ident)

            # ---------- weights: load, transpose to [din, dout], split hi/lo ----------
            whs = {}
            wls = {}
            with tc.tile_pool(name="wtmp", bufs=1) as wtmp:
                for name, W_hh in (("w1", W1_h), ("w2", W2_h), ("wk1", Wk1_h)):
                    raw = wtmp.tile([128, KT, D], FP, tag=f"raw_{name}")
                    # raw[p, j, d] = W[j*128 + p, d]
                    nc.sync.dma_start(
                        out=raw, in_=W_hh[:, :].rearrange("(j p) d -> p j d", p=128)
                    )
                    wT = wtmp.tile([128, KT, D], FP, tag=f"wT_{name}")
                    for k in range(KT):
                        for j in range(KT):
                            pt = trp.tile([128, 128], FP, tag="tr", name="pt_w")
                            nc.tensor.transpose(pt, raw[:, j, ts(k, 128)], ident)
                            if (k + j) % 2 == 0:
                                nc.vector.tensor_copy(out=wT[:, k, ts(j, 128)], in_=pt)
                            else:
                                nc.scalar.copy(wT[:, k, ts(j, 128)], pt)
                    wh = wp.tile([128, KT, D], FPR, tag=f"wh_{name}", name=f"wh_{name}")
                    nc.vector.tensor_copy(out=wh, in_=wT)
                    wl = wp.tile([128, KT, D], FPR, tag=f"wl_{name}", name=f"wl_{name}")
                    nc.vector.tensor_sub(wl, wT, wh.bitcast(FP))
                    whs[name] = wh
                    wls[name] = wl

                rawd = wtmp.tile([C, D], FP, tag="raw_wd")
                nc.sync.dma_start(out=rawd, in_=Wd_h[:, :])
                wdT = wtmp.tile([128, KT, C], FP, tag="wdT")
                for k in range(KT):
                    pt = trp.tile([128, C], FP, tag="tr", name="pt_wd")
                    nc.tensor.transpose(pt, rawd[:, ts(k, 128)], ident[:C, :C])
                    nc.vector.tensor_copy(out=wdT[:, k, :], in_=pt)
                wdh = wp.tile([128, KT, C], FPR, name="wdh")
                nc.vector.tensor_copy(out=wdh, in_=wdT)
                wdl = wp.tile([128, KT, C], FPR, name="wdl")
                nc.vector.tensor_sub(wdl, wdT, wdh.bitcast(FP))

                rawk2 = wtmp.tile([1, D], FP, tag="raw_wk2")
                nc.sync.dma_start(out=rawk2, in_=Wk2_h[:, :])
                wk2T = wtmp.tile([128, KT, 1], FP, tag="wk2T")
                for k in range(KT):
                    pt = trp.tile([128, 1], FP, tag="tr", name="pt_wk2")
                    nc.tensor.transpose(pt, rawk2[:, ts(k, 128)], ident[:1, :1])
                    nc.vector.tensor_copy(out=wk2T[:, k, :], in_=pt)
                wk2h = wp.tile([128, KT, 1], FPR, name="wk2h")
                nc.vector.tensor_copy(out=wk2h, in_=wk2T)
                wk2l = wp.tile([128, KT, 1], FPR, name="wk2l")
                nc.vector.tensor_sub(wk2l, wk2T, wk2h.bitcast(FP))

                b1sb = wp.tile([128, KT, 1], FP, tag="b1", name="b1sb")
                nc.sync.dma_start(
                    out=b1sb[:, :, 0], in_=b1_h[:].rearrange("(j p) -> p j", p=128)
                )
                bk1sb = wp.tile([128, KT, 1], FP, tag="bk1", name="bk1sb")
                nc.sync.dma_start(
                    out=bk1sb[:, :, 0], in_=bk1f_h[:].rearrange("(j p) -> p j", p=128)
                )
                bdsb = wp.tile([C, 1], FP, tag="bd", name="bdsb")
                nc.sync.dma_start(
                    out=bdsb, in_=bdf_h[:].rearrange("(c o) -> c o", o=1)
                )

            w1h, w1l = whs["w1"], wls["w1"]
            w2h, w2l = whs["w2"], wls["w2"]
            wk1h, wk1l = whs["wk1"], wls["wk1"]

            # persistent per-core accumulators
            scoresb = wp.tile([128, GT, S], FP, tag="scores")
            logits_dram = dp.tile([C, bc * S], FP)

            with (
                tc.tile_pool(name="io", bufs=3) as iop,
                tc.tile_pool(name="acts", bufs=1) as actp,
                tc.tile_pool(name="scr", bufs=4) as scrp,
                tc.tile_pool(name="small", bufs=2) as sp,
                tc.tile_pool(name="gating", bufs=1) as gp,
            ):
                # ---------- gating: top-16 + peaked softmax + weighted logits ----------
                def emit_gating(g):
                    pcnt = min(128, bc - g * 128)
                    sc_g = scoresb[:pcnt, g, :]                      # [pcnt, 64]
                    m8a = gp.tile([128, 8], FP, tag="m8a", name="m8a")[:pcnt, :]
                    nc.vector.max(out=m8a, in_=sc_g)                 # ranks 1-8
                    scratch = gp.tile([128, S], FP, tag="scratch", name="scr_g")[:pcnt, :]
                    nc.vector.match_replace(
                        out=scratch, in_to_replace=m8a, in_values=sc_g,
                        imm_value=NEG_BIG,
                    )
                    m8b = gp.tile([128, 8], FP, tag="m8b", name="m8b")[:pcnt, :]
                    nc.vector.max(out=m8b, in_=scratch)              # ranks 9-16
                    hard_g = gp.tile([128, S], FP, tag="hard", name="hard_t")[:pcnt, :]
                    nc.vector.tensor_scalar(
                        out=hard_g, in0=sc_g, scalar1=m8b[:, 7:8], scalar2=None,
                        op0=OP.is_ge,
                    )
                    negm = gp.tile([128, 1], FP, tag="negm", name="negm")[:pcnt, :]
                    nc.vector.tensor_scalar_mul(negm, m8a[:, 0:1], -INV_T)
                    e_g = gp.tile([128, S], FP, tag="e", name="e_t")[:pcnt, :]
                    esum = gp.tile([128, 1], FP, tag="esum", name="esum")[:pcnt, :]
                    nc.scalar.activation(
                        out=e_g, in_=sc_g, func=AF.Exp, bias=negm, scale=INV_T,
                        accum_out=esum,
                    )
                    gu = gp.tile([128, S], FP, tag="gu", name="gu")[:pcnt, :]
                    nc.vector.tensor_tensor(out=gu, in0=e_g, in1=hard_g, op=OP.mult)
                    ssum = gp.tile([128, 1], FP, tag="ssum", name="ssum")[:pcnt, :]
                    nc.vector.reduce_sum(out=ssum, in_=gu, axis=AX.X)
                    denom = gp.tile([128, 1], FP, tag="denom", name="denom")[:pcnt, :]
                    nc.vector.tensor_scalar(
                        out=denom, in0=esum, scalar1=1e-8, scalar2=None, op0=OP.mult
                    )
                    nc.vector.tensor_add(denom, denom, ssum)
                    rec = gp.tile([128, 1], FP, tag="rec", name="rec")[:pcnt, :]
                    nc.vector.reciprocal(rec, denom)
                    gate_g = gp.tile([128, S], FP, tag="gate", name="gate_t")[:pcnt, :]
                    nc.vector.tensor_scalar_mul(gate_g, gu, rec)

                    nc.sync.dma_start(out=gate_h[ds(g * 128, pcnt), :], in_=gate_g)
                    nc.sync.dma_start(out=hard_h[ds(g * 128, pcnt), :], in_=hard_g)

                    # x[b, c] = sum_s gate[b, s] * logits[c, b*S + s]
                    xo = gp.tile([128, C], FP, tag="xo", name="xo")[:pcnt, :]
                    for c in range(C):
                        lc = gp.tile([128, S], FP, tag="lc", name="lc")[:pcnt, :]
                        nc.sync.dma_start(
                            out=lc, in_=logits_dram[c, ds(g * 128 * S, pcnt * S)]
                        )
                        tmp = gp.tile([128, S], FP, tag="xtmp", name="xtmp")[:pcnt, :]
                        nc.vector.tensor_tensor(out=tmp, in0=gate_g, in1=lc, op=OP.mult)
                        nc.vector.reduce_sum(out=xo[:, c : c + 1], in_=tmp, axis=AX.X)
                    nc.sync.dma_start(out=x_h[ds(g * 128, pcnt), :], in_=xo)

                # ---------- main loop over token chunks ----------
                # (reps>1 repeats the whole compute for wall-clock delta
                #  timing; outputs are rewritten identically each rep)
                import itertools
                for _rep, ct in itertools.product(range(reps), range(nch)):
                    x_tm = iop.tile([128, KT, D], FP, tag="x_tm")
                    # x_tm[p, t, d] = slots_flat[ct*TT + t*128 + p, d]
                    nc.sync.dma_start(
                        out=x_tm,
                        in_=flat[ds(ct * TT, TT), :].rearrange(
                            "(t p) d -> p t d", p=128
                        ),
                    )
                    # transpose to feature-major, split hi/lo while evicting
                    xh = iop.tile([128, KT, TT], FPR, tag="xh")
                    xl = iop.tile([128, KT, TT], FPR, tag="xl")
                    for k in range(KT):
                        pt = trp.tile([128, TT], FP, tag="tr", name="pt_x")
                        for t in range(TT // 128):
                            nc.tensor.transpose(
                                pt[:, ts(t, 128)], x_tm[:, t, ts(k, 128)], ident
                            )
                        nc.scalar.copy(xh[:, k, :], pt)
                        nc.vector.tensor_sub(
                            xl[:, k, :], pt, xh[:, k, :].bitcast(FP)
                        )

                    def layer3(out_hi, out_lo, wh_, wl_, rh_, rl_, bias, relu):
                        """out = act(W @ r + bias); r/W as hi/lo fp32r splits,
                        out written as hi/lo fp32r split (exact residual)."""
                        for j in range(KT):
                            pm = mmp.tile([128, TT], FP, tag="mm", name="pm")
                            n = 0
                            for k in range(KT):
                                for lh_ap, rh_ap in (
                                    (wh_[:, k, ts(j, 128)], rh_[:, k, :]),
                                    (wl_[:, k, ts(j, 128)], rh_[:, k, :]),
                                    (wh_[:, k, ts(j, 128)], rl_[:, k, :]),
                                ):
                                    nc.tensor.matmul(
                                        pm, lhsT=lh_ap, rhs=rh_ap,
                                        start=(n == 0), stop=(n == 3 * KT - 1),
                                    )
                                    n += 1
                            if relu:
                                scratch = scrp.tile(
                                    [128, TT], FP, tag="scratch", name="scratch"
                                )
                                nc.scalar.activation(
                                    out=scratch, in_=pm, func=AF.Relu,
                                    bias=bias[:, j, :],
                                )
                                nc.vector.tensor_copy(out=out_hi[:, j, :], in_=scratch)
                                nc.vector.tensor_sub(
                                    out_lo[:, j, :], scratch,
                                    out_hi[:, j, :].bitcast(FP),
                                )
                            else:
                                nc.scalar.copy(out_hi[:, j, :], pm)
                                nc.vector.tensor_sub(
                                    out_lo[:, j, :], pm, out_hi[:, j, :].bitcast(FP)
                                )

                    # layer 1: h = relu(W1 @ x + b1)
                    hh = actp.tile([128, KT, TT], FPR, tag="hh")
                    hl = actp.tile([128, KT, TT], FPR, tag="hl")
                    layer3(hh, hl, w1h, w1l, xh, xl, b1sb, relu=True)

                    # layer 2: s = W2 @ h   (b2 folded into bk1f/bdf)
                    sh = actp.tile([128, KT, TT], FPR, tag="sh")
                    sl = actp.tile([128, KT, TT], FPR, tag="sl")
                    layer3(sh, sl, w2h, w2l, hh, hl, None, relu=False)

                    # layer 3: hk = relu(Wk1 @ s + bk1f)
                    hkh = actp.tile([128, KT, TT], FPR, tag="hkh")
                    hkl = actp.tile([128, KT, TT], FPR, tag="hkl")
                    layer3(hkh, hkl, wk1h, wk1l, sh, sl, bk1sb, relu=True)

                    # logits head (3-pass split): [C, TT] = Wd @ s + bdf
                    pl = hp.tile([C, TT], FP, tag="lg", name="pl")
                    n = 0
                    for k in range(KT):
                        for lh_ap, rh_ap in (
                            (wdh[:, k, :], sh[:, k, :]),
                            (wdl[:, k, :], sh[:, k, :]),
                            (wdh[:, k, :], sl[:, k, :]),
                        ):
                            nc.tensor.matmul(
                                pl, lhsT=lh_ap, rhs=rh_ap,
                                start=(n == 0), stop=(n == 3 * KT - 1),
                            )
                            n += 1
                    lg_sb = sp.tile([C, TT], FP, tag="lg_sb")
                    nc.scalar.activation(out=lg_sb, in_=pl, func=AF.Identity, bias=bdsb)
                    nc.sync.dma_start(out=logits_dram[:, ds(ct * TT, TT)], in_=lg_sb)

                    # keep-score head (3-pass split): [1, TT] = Wk2 @ hk
                    psc = hp.tile([1, TT], FP, tag="sc", name="psc")
                    n = 0
                    for k in range(KT):
                        for lh_ap, rh_ap in (
                            (wk2h[:, k, :], hkh[:, k, :]),
                            (wk2l[:, k, :], hkh[:, k, :]),
                            (wk2h[:, k, :], hkl[:, k, :]),
                        ):
                            nc.tensor.matmul(
                                psc, lhsT=lh_ap, rhs=rh_ap,
                                start=(n == 0), stop=(n == 3 * KT - 1),
                            )
                            n += 1
                    sc_sb = sp.tile([1, TT], FP, tag="sc_sb")
                    nc.scalar.copy(sc_sb, psc)
                    # reshape [1, 512] -> [8 rows, 64 slots] at batch-row partitions
                    base = (ct * ROWS_PER_CHUNK) % 128
                    g1 = (ct * ROWS_PER_CHUNK) // 128
                    nc.sync.dma_start(
                        out=scoresb[ds(base, ROWS_PER_CHUNK), g1, :], in_=sc_sb
                    )
                    rows_done = (ct + 1) * ROWS_PER_CHUNK
                    if rows_done % 128 == 0 and _rep == reps - 1:
                        emit_gating(rows_done // 128 - 1)


                for g in range((bc * S // TT) * ROWS_PER_CHUNK // 128, GT):
                    emit_gating(g)

    nc.compile()
    return nc


_NC_CACHE = {}


def _get_program(bc):
    if bc not in _NC_CACHE:
        _NC_CACHE[bc] = build_program(bc)
    return _NC_CACHE[bc]


def make_weight_inputs(W1, b1, W2, b2, Wd, bd, Wk1, bk1, Wk2):
    """Device weight dict with b2 folded into downstream biases (fp64 fold)."""
    bk1f = (Wk1.astype(np.float64) @ b2.astype(np.float64) + bk1).astype(np.float32)
    bdf = (Wd.astype(np.float64) @ b2.astype(np.float64) + bd).astype(np.float32)
    return {
        "W1": np.ascontiguousarray(W1, np.float32),
        "b1": np.ascontiguousarray(b1, np.float32),
        "W2": np.ascontiguousarray(W2, np.float32),
        "Wd": np.ascontiguousarray(Wd, np.float32),
        "bdf": np.ascontiguousarray(bdf, np.float32),
        "Wk1": np.ascontiguousarray(Wk1, np.float32),
        "bk1f": np.ascontiguousarray(bk1f, np.float32),
        "Wk2": np.ascontiguousarray(Wk2, np.float32),
    }


def kernel(slots, W1, b1, W2, b2, Wd, bd, Wk1, bk1, Wk2, bk2):
    from concourse.bass_utils import run_bass_kernel_spmd

    B = slots.shape[0]
    assert B % N_CORES == 0
    bc = B // N_CORES
    nc = _get_program(bc)

    weights = make_weight_inputs(W1, b1, W2, b2, Wd, bd, Wk1, bk1, Wk2)
    in_maps = []
    for i in range(N_CORES):
        m = dict(weights)
        m["slots"] = np.ascontiguousarray(slots[i * bc : (i + 1) * bc], np.float32)
        in_maps.append(m)

    res = run_bass_kernel_spmd(nc, in_maps, core_ids=list(range(N_CORES)))
    x = np.concatenate([r["x"] for r in res.results], axis=0)
    gate = np.concatenate([r["gate"] for r in res.results], axis=0)
    hard = np.concatenate([r["hard"] for r in res.results], axis=0)
    return x, gate, hard


if __name__ == "__main__":
    import time

    t0 = time.time()
    nc = build_program(int(sys.argv[1]) if len(sys.argv) > 1 else BC)
    print(f"traced in {time.time() - t0:.1f}s")
